# revision 42
# baseline (speedup 1.0000x reference)
"""Trainium2 Bass kernel for batched filtfilt band-pass filtering (tensorpac-style).

Math: scipy-style filtfilt with FIR taps b is (exactly) a single convolution of
the odd-extended input with the autocorrelation of b, evaluated on the interior:

    out[n] = sum_d A[d] * ext[P + n + d],   d in [-(t-1), t-1]
    A[d]   = sum_i b[i] * b[i+d]            (t = effective tap count)

provided padlen P >= t-1 (true here: P = 512, t <= 513). The left "lfilter_zi"
constant extension and the right-edge extension of the backward pass never reach
the retained [P, P+L) window, so the equivalence is exact (verified to 1e-16).

Device mapping (per core, sequence-parallel over 8 cores):
  - each core owns 2048 output positions x all 128 batches; its input is a
    (3072, 128) slice of ext^T (position-major) covering the 2x512 halo,
    shipped fp16 in the SBUF-native [partition, h-block, batch] layout.
  - out[r, (j,b)] tiles (128 positions x 4 pos-blocks x 128 batches) accumulate
    in fp32 PSUM via K=128 fp16 matmuls: lhsT = 128x128 banded-Toeplitz blocks
    of A (host-precomputed fp16 constants), rhs = 512-wide slices of ext^T.
  - per band, the number of Toeplitz blocks adapts to the true tap support
    (Q = ceil((2t+126)/128)); a half-block-shifted copy of ext^T (E64, built
    on-device from E via two partition-shifted SBUF->SBUF DMAs) lets short
    bands cover their diagonal band with Q = minimal block count.
  - loop is band-outer (large/small-Q bands interleaved) so the per-band
    constant stream (2.1 MB) overlaps the matmul phase and the PSUM drain
    stays smooth; PSUM tiles drain via a DVE/ACT split copy that also casts
    to fp16, and each band leaves as one contiguous 0.5 MB DMA on the SP
    HWDGE ring (the last band streams per group to shorten the tail).
  - dummy warm-up matmuls run while the first inputs land so the PE HAM
    clock-gate is released before real work starts.
"""

import os

import numpy as np

import concourse.mybir as mybir
from concourse import bacc
from concourse.tile import TileContext
from concourse.bass_utils import run_bass_kernel_spmd

F32 = mybir.dt.float32
F16 = mybir.dt.float16

B = 128          # batch
L = 16384        # sequence length
P = 512          # padlen (= TAPS - 1)
NB = 20          # bands
N_CORES = 8
LC = L // N_CORES            # 2048 output positions per core
GROUPS = LC // 512           # 4 groups of 512 positions
EXT_ROWS = LC + 2 * P        # 3072 ext rows per core (halo included)
H_E = EXT_ROWS // 128        # 24 aligned 128-row blocks
H_E64 = (EXT_ROWS - 128) // 128  # 23 half-shifted blocks (rows 64 + 128h + p)
N_WARM = 14                  # dummy matmuls to warm the PE HAM during input DMA

LAST_RESULT = None  # BassKernelResults of the most recent run (for test harness)

_program_cache: dict = {}


def _band_plan(kernels: np.ndarray):
    """Per-band tap support -> (t, Q, s, use64, h_base) block plan.

    Block q covers ext rows m = n0 + P - s + 128q + kk (kk = partition), so
    diagonal d = 128q + kk - s - r. Coverage of d in [-(t-1), t-1] for every
    r in [0,128) requires s >= t-1 and s <= 128Q - 127 - t. s is the smallest
    multiple of 64 >= t-1; s % 128 == 64 uses the half-shifted E64 copy.
    """
    plan = []
    for k in range(kernels.shape[0]):
        nz = np.nonzero(kernels[k])[0]
        t = int(nz[-1]) + 1 if nz.size else 1
        assert t - 1 <= P, f"band {k}: taps {t} exceed padlen {P}"
        q_cnt = (2 * t + 126 + 127) // 128
        s = 64 * ((t - 1 + 63) // 64) if t > 1 else 0
        assert s >= t - 1 and s <= 128 * q_cnt - 127 - t, (k, t, q_cnt, s)
        use64 = (s % 128) == 64
        if use64:
            h_base = (P - 64 - s) // 128
        else:
            h_base = (P - s) // 128
        assert h_base >= 0
        plan.append((t, q_cnt, s, use64, h_base))
    return plan


def _band_order(plan):
    """First a small aligned band (so PE work starts on a partial E), LAST
    the biggest band: small-Q bands drain slower than they matmul (drain
    ~1.9us/band vs Q=2 matmul 1.7us), so each must sit next to a big band
    that gives DVE/ACT slack - ending on the biggest band lets every
    earlier drain catch up and keeps the PE gap-free to the end. E64 bands
    are kept out of the first two slots to cover the E64 build latency."""
    by_q = sorted(range(len(plan)), key=lambda k: (-plan[k][1], plan[k][3]))
    small_aligned = [k for k in by_q if not plan[k][3]]
    first = small_aligned[-1] if small_aligned else by_q[0]
    last = by_q[0] if by_q[0] != first else by_q[1]
    rest = [k for k in by_q if k != first and k != last]
    order, lo, hi = [first], 0, len(rest) - 1
    while lo <= hi:
        order.append(rest[lo]); lo += 1
        if lo <= hi:
            order.append(rest[hi]); hi -= 1
    order.append(last)
    return order


def _toeplitz_blocks(kernels: np.ndarray, plan, order):
    """Stacked lhsT blocks in SBUF-native layout: (128, NBLK, 128) fp16,
    [kk, block, r] with the contraction dim kk on axis 0. Blocks are laid
    out in BAND-ORDER (slot-major) so the whole constant stream is one or
    two contiguous DMAs that land in the order the matmuls consume them."""
    nblk = sum(p[1] for p in plan)
    out = np.zeros((128, nblk, 128), np.float16)
    kk = np.arange(128)[:, None]
    rr = np.arange(128)[None, :]
    i = 0
    for k in order:
        t, q_cnt, s, _use64, _hb = plan[k]
        bk = kernels[k][:t].astype(np.float64)
        acorr = np.correlate(bk, bk, mode="full")  # length 2t-1, center t-1
        a_full = np.zeros(2 * P + 1, np.float64)
        a_full[P - (t - 1) : P + t] = acorr
        for q in range(q_cnt):
            d = 128 * q - s + kk - rr
            valid = (d >= -(t - 1)) & (d <= t - 1)
            blk = np.where(valid, a_full[np.clip(d + P, 0, 2 * P)], 0.0)
            out[:, i, :] = blk.astype(np.float16)
            i += 1
    return out


def _build_program(plan_key):
    """Compile the SPMD program for a given block structure. Cached."""
    if plan_key in _program_cache:
        return _program_cache[plan_key]

    plan = list(plan_key)
    order = _band_order(plan)
    # block offsets are SLOT-major (band-order), matching _toeplitz_blocks
    slot_offsets = np.cumsum([0] + [plan[k][1] for k in order]).tolist()
    nblk = slot_offsets[-1]
    # out-DMA taper: leading slots ship in 4-band chunks (fewer ~0.6us
    # triggers on the sequencers), the last 8 slots ship individually the
    # moment they drain (4KB/partition descriptors, alternating rings) so
    # nothing big queues at the kernel tail
    chunk_sizes = (4, 4, 4, 1, 1, 1, 1, 1, 1, 1, 1)
    assert sum(chunk_sizes) == NB
    out_chunks = []
    si = 0
    for n in chunk_sizes:
        out_chunks.append((si, n))
        si += n

    nc = bacc.Bacc("TRN2", target_bir_lowering=False, debug=False,
                   num_devices=N_CORES)
    # host-permuted ext^T slice: [p, h, b] fp16 (SBUF-native layout)
    ext_in = nc.declare_dram_parameter("ext", [128, H_E, B], F16, isOutput=False)
    lhs_in = nc.declare_dram_parameter("lhs", [128, nblk, 128], F16,
                                       isOutput=False)
    out_t = nc.declare_dram_parameter("out", [NB, 128, GROUPS * 512], F16,
                                      isOutput=True)

    with TileContext(nc) as tc:
        with (
            tc.tile_pool(name="consts", bufs=1) as cpool,
            tc.tile_pool(name="psum", bufs=8, space="PSUM") as ppool,
            tc.tile_pool(name="ostage", bufs=6) as opool,
        ):
            E = cpool.tile([128, H_E * 128], F16)
            E64 = cpool.tile([128, H_E64 * 128], F16)
            Lw = cpool.tile([128, nblk * 128], F16)
            warm = cpool.tile([128, 256], F16)
            wps = ppool.tile([128, 512], F32, tag="ps")

            # PE warm-up during the input DMAs: harmless matmuls on a zeroed
            # tile keep the HAM busy window alive so real matmuls start warm.
            # memset on DVE: nc.any would pick GpSimd, whose multi-us engine
            # cold-start delays the whole warm-up chain.
            nc.vector.memset(warm[:], 0.0)
            for w in range(N_WARM):
                nc.tensor.matmul(wps[:, 0:256], warm[:, :128], warm[:],
                                 start=True, stop=True)

            # E in 3 asymmetric chunks: the first covers exactly the h-blocks
            # the first band's g=0 matmuls touch, so real matmuls start ASAP
            t0_, q0_, _s0, _u0, hb0 = plan[order[0]]
            chunk0 = min(hb0 + q0_ + 3, 15) * 128
            e_flat = ext_in[:].rearrange("p h b -> p (h b)")
            chunk = 15 * 128
            nc.sync.dma_start(out=E[:, 0:chunk0], in_=e_flat[:, 0:chunk0])
            if chunk0 < chunk:
                nc.sync.dma_start(out=E[:, chunk0:chunk], in_=e_flat[:, chunk0:chunk])
            nc.sync.dma_start(out=E[:, chunk:], in_=e_flat[:, chunk:])
            # E64[p, h] = ext rows (64 + 128h + p), built on device from E.
            # The sem-wait of these triggers head-of-line blocks the HWDGE
            # ring, which (deliberately) gives E exclusive DMA bandwidth.
            e3 = E[:].rearrange("p (h b) -> p h b", b=B)
            e643 = E64[:].rearrange("p (h b) -> p h b", b=B)
            nc.sync.dma_start(out=e643[0:64, :, :], in_=e3[64:128, 0:H_E64, :])
            nc.sync.dma_start(out=e643[64:128, :, :], in_=e3[0:64, 1 : H_E64 + 1, :])

            # constants are pre-ordered slot-major on the host, so the 2.1 MB
            # stream is FOUR contiguous graduated DMAs on the ACT HWDGE ring
            # (4 trigger issues instead of 20). Graduation matters because a
            # DMA completes as one unit: each chunk must land before the MM
            # stream reaches its first slot, so early chunks are small.
            for lo, hi in ((0, 2), (2, 6), (6, 12), (12, NB)):
                oa, ob_ = slot_offsets[lo], slot_offsets[hi]
                nc.scalar.dma_start(
                    out=Lw[:, oa * 128 : ob_ * 128].rearrange(
                        "kk (i r) -> kk i r", r=128
                    ),
                    in_=lhs_in[:, oa:ob_, :],
                )

            # chunk tiles staged for the tapered multi-band out-DMAs
            chunk_of_slot = {}
            for ci, (s0, n) in enumerate(out_chunks):
                for j in range(n):
                    chunk_of_slot[s0 + j] = ci
            chunk_tiles = {}

            for si, k in enumerate(order):
                t, q_cnt, s, use64, h_base = plan[k]
                o = slot_offsets[si]
                src = E64 if use64 else E
                h_max = H_E64 if use64 else H_E
                ci = chunk_of_slot[si]
                s0, n = out_chunks[ci]
                if ci not in chunk_tiles:
                    chunk_tiles[ci] = opool.tile(
                        [128, n * GROUPS * 512], F16, name="obc",
                        tag=f"obc{n}", bufs=(2 if n > 1 else 3),
                    )
                ob = chunk_tiles[ci]
                obase = (si - s0) * GROUPS * 512
                if k == order[0]:
                    # group-outer for the first band only: its g=0 matmuls
                    # need just the first few E h-blocks, so PE work starts
                    # while the rest of E is still in flight
                    for g in range(GROUPS):
                        ps = ppool.tile([128, 512], F32)
                        for qi in range(q_cnt):
                            h0 = 4 * g + h_base + qi
                            assert 0 <= h0 and h0 + 4 <= h_max, (k, g, qi, h0)
                            nc.tensor.matmul(
                                ps[:],
                                Lw[:, (o + qi) * 128 : (o + qi + 1) * 128],
                                src[:, h0 * 128 : h0 * 128 + 512],
                                start=(qi == 0),
                                stop=(qi == q_cnt - 1),
                            )
                        base = obase + g * 512
                        if g % 2 == 0:
                            nc.vector.tensor_copy(ob[:, base : base + 512], ps[:])
                        else:
                            nc.scalar.copy(ob[:, base : base + 512], ps[:])
                else:
                    # qi-outer: the 4 groups' PSUM banks accumulate in
                    # lockstep so each lhsT block is (re)used by 4 back-to-
                    # back matmuls - the weight (re)load amortizes across
                    # the group sweep instead of being paid per matmul
                    pss = []
                    for g in range(GROUPS):
                        ps_g = ppool.tile([128, 512], F32, name="ps", tag="ps")
                        pss.append(ps_g)
                    for qi in range(q_cnt):
                        w = Lw[:, (o + qi) * 128 : (o + qi + 1) * 128]
                        for g in range(GROUPS):
                            h0 = 4 * g + h_base + qi
                            assert 0 <= h0 and h0 + 4 <= h_max, (k, g, qi, h0)
                            nc.tensor.matmul(
                                pss[g][:],
                                w,
                                src[:, h0 * 128 : h0 * 128 + 512],
                                start=(qi == 0),
                                stop=(qi == q_cnt - 1),
                            )
                    # whole-tile single-engine drains: DVE takes groups 0-2,
                    # ACT group 3. Each PSUM tile is freed by ONE copy (no
                    # split-copy rendezvous), instruction count halves, and
                    # the per-band drain wall (~1.7us) stays under even a
                    # Q=2 band's matmul time so the PE never waits on banks.
                    for g in range(GROUPS):
                        base = obase + g * 512
                        if g < 3:
                            nc.vector.tensor_copy(ob[:, base : base + 512],
                                                  pss[g][:])
                        else:
                            nc.scalar.copy(ob[:, base : base + 512], pss[g][:])
                # ship each completed chunk as ONE contiguous DMA (out_t is
                # slot-major; the host unscrambles), alternating rings
                # chunk-by-chunk. Keep the partition dim outermost on BOTH
                # sides of the AP - a leading free dim over SBUF partitions
                # generates descriptors the DGE cannot execute.
                if si == s0 + n - 1:
                    eng = nc.sync if ci % 2 == 0 else nc.scalar
                    eng.dma_start(
                        out=out_t[s0 : s0 + n].rearrange("i p f -> p i f"),
                        in_=ob[:].rearrange("p (i f) -> p i f", i=n),
                    )

    nc.compile()
    _program_cache[plan_key] = nc
    return nc


def _maybe_register_trace_hook():
    """Best-effort registration of the axon NTFF profile hook (profiling only;
    harmless no-op if unavailable)."""
    try:
        import sys
        import types

        import antenv

        if getattr(antenv, "axon_hooks", None) is not None:
            return
        from trn_agent_boot.trn_boot import _ntff_profile_via_ctypes

        hooks = types.ModuleType("antenv.axon_hooks")
        hook = _ntff_profile_via_ctypes("/opt/axon/libaxon_pjrt.so")
        hooks.get_axon_ntff_profile_hook = lambda: hook
        hooks.set_axon_ntff_profile_hook = lambda h: None
        antenv.axon_hooks = hooks
        sys.modules["antenv.axon_hooks"] = hooks
    except Exception:
        pass


def kernel(x: np.ndarray, kernels: np.ndarray, padlen) -> np.ndarray:
    global LAST_RESULT
    x = np.asarray(x, dtype=np.float32)
    kernels = np.asarray(kernels, dtype=np.float32)
    assert x.shape == (B, 1, L) and kernels.shape[0] == NB
    assert int(padlen) == P

    plan = _band_plan(kernels)
    plan_key = tuple(plan)
    nc = _build_program(plan_key)

    order = _band_order(plan)
    lhs = np.ascontiguousarray(_toeplitz_blocks(kernels, plan, order))

    # odd extension + transpose to position-major (ext^T), fp16
    x2d = x[:, 0, :]
    left = 2.0 * x2d[:, :1] - x2d[:, 1 : P + 1][:, ::-1]
    right = 2.0 * x2d[:, -1:] - x2d[:, -P - 1 : -1][:, ::-1]
    ext_t = np.concatenate([left, x2d, right], axis=1).T.astype(np.float16)

    in_maps = []
    for c in range(N_CORES):
        sl = ext_t[c * LC : c * LC + EXT_ROWS]  # (3072, B)
        # SBUF-native layout [p, h, b]: row (128h + p) -> [p, h]
        slp = np.ascontiguousarray(
            sl.reshape(H_E, 128, B).transpose(1, 0, 2)
        )
        in_maps.append({"ext": slp, "lhs": lhs})

    trace = bool(os.environ.get("KERNEL_TRACE"))
    if trace:
        _maybe_register_trace_hook()
    res = run_bass_kernel_spmd(nc, in_maps, list(range(N_CORES)), trace=trace)
    LAST_RESULT = res

    out = np.empty((B, 1, NB, L), np.float32)
    band_of_slot = np.asarray(order)  # out rows are slot-major on device
    for c in range(N_CORES):
        dev = res.results[c]["out"].astype(np.float32).reshape(NB, 128, GROUPS, 4, 128)
        # dev[slot, r, g, j, b] -> out[b, 0, order[slot], c*LC + 512g + 128j + r]
        arr = dev.transpose(4, 0, 2, 3, 1).reshape(B, NB, LC)
        out[:, 0, band_of_slot, c * LC : (c + 1) * LC] = arr
    return out



# revision 44
# speedup vs baseline: 1.0241x; 1.0241x over previous
"""Trainium2 Bass kernel: filtfilt band-pass, 4 longest bands decimated.

out = autocorr(b) (*) odd-ext(x) per band. Two device paths:
- DIRECT (16 bands, t<=154): banded-Toeplitz fp16 matmuls, qi-outer over 4
  per-group [128,512] PSUM tiles (weight reuse x4), split DVE/ACT drains.
- DECIMATED (4 bands, t in [193,513]): A's response is |B(f)|^2 (stopband
  ~-106 dB), content < 32 Hz, so compute at 1/8 rate and interpolate:
    od[u]  = sum_d A[d] ext[8u+d]   (stride-8 Toeplitz, N=384 matmuls)
    out[n] = sum_u I[n-8u] od[u]    (4 shared 128x128 interp classes)
  Validated on host vs fp64: ~4e-4 per band.
The interp stage's 16 w-window matmuls (one per 128 output positions, all 4
bands per matmul) are interleaved between direct bands so their PSUM drains
dilute into the direct-band drain stream instead of bunching.
"""

import os

import numpy as np

import concourse.mybir as mybir
from concourse import bacc
from concourse.tile import TileContext
from concourse.bass_utils import run_bass_kernel_spmd

F32 = mybir.dt.float32
F16 = mybir.dt.float16

B = 128
L = 16384
P = 512
NB = 20
N_CORES = 8
LC = L // N_CORES
GROUPS = LC // 512
EXTO = 1024
EXT_ROWS = LC + 2 * EXTO     # 4096
H_E = EXT_ROWS // 128        # 32
H_E64 = H_E - 1              # 31 valid half-shifted blocks (tile padded to 32)
DEC = (0, 1, 2, 3)           # t = 513, 385, 257, 193
DIR = tuple(k for k in range(NB) if k not in DEC)
DD = 8
J = 160
N_WARM = 12
CLS = (64, 80, 32, 48)       # interp lhsT class for w % 4
NKB = len(DEC)

LAST_RESULT = None
_program_cache: dict = {}


def _dir_plan(kernels):
    plan = {}
    for k in DIR:
        nz = np.nonzero(kernels[k])[0]
        t = int(nz[-1]) + 1 if nz.size else 1
        q_cnt = (2 * t + 126 + 127) // 128
        s = 64 * ((t - 1 + 63) // 64) if t > 1 else 0
        assert s >= t - 1 and s <= 128 * q_cnt - 127 - t, (k, t, q_cnt, s)
        use64 = (s % 128) == 64
        h_base = (EXTO - 64 - s) // 128 if use64 else (EXTO - s) // 128
        plan[k] = (t, q_cnt, s, use64, h_base)
    return plan


def _dec_plan(kernels):
    plan = {}
    for k in DEC:
        nz = np.nonzero(kernels[k])[0]
        t = int(nz[-1]) + 1 if nz.size else 1
        s8 = 64 * ((t - 1 + 63) // 64)
        qd = -(-(t + s8 + 1017) // 128)
        base0 = 512 - s8
        assert base0 >= 0 and base0 % 64 == 0
        use64 = (base0 % 128) == 64
        h0 = (base0 - 64) // 128 if use64 else base0 // 128
        assert h0 + qd - 1 <= 15, (k, t, h0, qd)
        plan[k] = (t, s8, qd, use64, h0)
    return plan


def _orders(dirplan):
    """16 direct bands: first must be 128-aligned (E); interleave big/small
    Q so drains stay smooth."""
    by_q = sorted(DIR, key=lambda k: (-dirplan[k][1], dirplan[k][3]))
    aligned = [k for k in by_q if not dirplan[k][3]]
    # first TWO slots aligned: the on-device E64 build (~5us) must complete
    # before the first use64 band starts
    head = [aligned[-1], aligned[0]]
    rest = [k for k in by_q if k not in head]
    d_ord, lo, hi = list(head), 0, len(rest) - 1
    while lo <= hi:
        d_ord.append(rest[lo]); lo += 1
        if lo <= hi:
            d_ord.append(rest[hi]); hi -= 1
    return d_ord, list(DEC)


def _autocorr_full(bk, t):
    a = np.correlate(bk[:t].astype(np.float64), bk[:t].astype(np.float64), "full")
    a_full = np.zeros(2 * EXTO + 1, np.float64)
    a_full[EXTO - (t - 1) : EXTO + t] = a
    return a_full


def _dir_blocks(kernels, dirplan, d_ord):
    nblk = sum(dirplan[k][1] for k in d_ord)
    out = np.zeros((128, nblk, 128), np.float16)
    kk = np.arange(128)[:, None]
    rr = np.arange(128)[None, :]
    i = 0
    for k in d_ord:
        t, q_cnt, s, _u, _hb = dirplan[k]
        a_full = _autocorr_full(kernels[k], t)
        for q in range(q_cnt):
            d = 128 * q - s + kk - rr
            blk = np.where((d >= -(t - 1)) & (d <= t - 1),
                           a_full[np.clip(d + EXTO, 0, 2 * EXTO)], 0.0)
            out[:, i, :] = blk.astype(np.float16)
            i += 1
    return out


def _dec_blocks(kernels, decplan, dec_ord):
    nblk = sum(decplan[k][2] for k in dec_ord)
    out = np.zeros((128, nblk, 128), np.float16)
    kk = np.arange(128)[:, None]
    rr = np.arange(128)[None, :]
    i = 0
    for k in dec_ord:
        t, s8, qd, _u, _h0 = decplan[k]
        a_full = _autocorr_full(kernels[k], t)
        for qi in range(qd):
            d = 128 * qi + kk - s8 - 8 * rr
            blk = np.where((d >= -(t - 1)) & (d <= t - 1),
                           a_full[np.clip(d + EXTO, 0, 2 * EXTO)], 0.0)
            out[:, i, :] = blk.astype(np.float16)
            i += 1
    return out


def _interp_blocks():
    j = np.arange(-J, J + 1)
    I = np.sinc(j / DD) * np.kaiser(2 * J + 1, 9.0)
    kk = np.arange(128)[:, None]
    rr = np.arange(128)[None, :]
    out = np.zeros((128, 4, 128), np.float16)
    for ci, c in enumerate(CLS):
        jj = 8 * c + rr - 8 * kk
        blk = np.where(np.abs(jj) <= J, I[np.clip(jj + J, 0, 2 * J)], 0.0)
        out[:, ci, :] = blk.astype(np.float16)
    return out


def _build_program(plan_key):
    if plan_key in _program_cache:
        return _program_cache[plan_key]
    dirplan, decplan = plan_key
    dirplan, decplan = dict(dirplan), dict(decplan)
    d_ord, dec_ord = _orders(dirplan)

    d_off = np.cumsum([0] + [dirplan[k][1] for k in d_ord]).tolist()
    dec_off = np.cumsum([0] + [decplan[k][2] for k in dec_ord]).tolist()
    nblk_d = d_off[-1]
    nblk_dec = dec_off[-1]

    nc = bacc.Bacc("TRN2", target_bir_lowering=False, debug=False,
                   num_devices=N_CORES)
    ext_in = nc.declare_dram_parameter("ext", [128, H_E, B], F16, isOutput=False)
    lhs_in = nc.declare_dram_parameter("lhs", [128, nblk_d, 128], F16,
                                       isOutput=False)
    lhsd_in = nc.declare_dram_parameter("lhsd", [128, nblk_dec, 128], F16,
                                        isOutput=False)
    lhsi_in = nc.declare_dram_parameter("lhsi", [128, 4, 128], F16,
                                        isOutput=False)
    out_t = nc.declare_dram_parameter("out", [NB, 128, GROUPS * 512], F16,
                                      isOutput=True)

    with TileContext(nc) as tc:
        with (
            tc.tile_pool(name="consts", bufs=1) as cpool,
            tc.tile_pool(name="psum", bufs=8, space="PSUM") as ppool,
            tc.tile_pool(name="ostage", bufs=1) as opool,
        ):
            E = cpool.tile([128, H_E * 128], F16)
            E64 = cpool.tile([128, H_E * 128], F16)  # block 31 unwritten pad
            Lw = cpool.tile([128, nblk_d * 128], F16)
            Lwd = cpool.tile([128, nblk_dec * 128], F16)
            Lwi = cpool.tile([128, 4 * 128], F16)
            warm = cpool.tile([128, 256], F16)
            odA = cpool.tile([128, 3 * NKB * 128], F16)
            odB = cpool.tile([128, 2 * NKB * 128], F16)
            obi = cpool.tile([128, NKB * 16 * 128], F16)  # kb-major interp out

            nc.vector.memset(warm[:], 0.0)
            wps = ppool.tile([128, 512], F32, name="ps", tag="ps")
            for _ in range(N_WARM):
                nc.tensor.matmul(wps[:, 0:256], warm[:, :128], warm[:],
                                 start=True, stop=True)

            t0_, q0_, _s0, _u0, hb0 = dirplan[d_ord[0]]
            assert not _u0, "first direct band must be 128-aligned (uses E)"
            chunk0 = min(hb0 + q0_ + 3, H_E) * 128
            e_flat = ext_in[:].rearrange("p h b -> p (h b)")
            nc.sync.dma_start(out=E[:, 0:chunk0], in_=e_flat[:, 0:chunk0])
            nc.sync.dma_start(out=E[:, chunk0:], in_=e_flat[:, chunk0:])
            e3 = E[:].rearrange("p (h b) -> p h b", b=B)
            e643 = E64[:].rearrange("p (h b) -> p h b", b=B)
            nc.sync.dma_start(out=e643[0:64, 0:H_E64, :], in_=e3[64:128, 0:H_E64, :])
            nc.sync.dma_start(out=e643[64:128, 0:H_E64, :],
                              in_=e3[0:64, 1 : H_E64 + 1, :])

            # constants on the ACT ring, graduated so prefixes land early
            ogd = d_off[2]
            nc.scalar.dma_start(
                out=Lw[:, 0 : ogd * 128].rearrange("kk (i r) -> kk i r", r=128),
                in_=lhs_in[:, 0:ogd, :])
            nc.scalar.dma_start(
                out=Lw[:, ogd * 128 :].rearrange("kk (i r) -> kk i r", r=128),
                in_=lhs_in[:, ogd:, :])
            og = dec_off[2]
            nc.scalar.dma_start(
                out=Lwd[:, 0 : og * 128].rearrange("kk (i r) -> kk i r", r=128),
                in_=lhsd_in[:, 0:og, :])
            nc.scalar.dma_start(
                out=Lwd[:, og * 128 :].rearrange("kk (i r) -> kk i r", r=128),
                in_=lhsd_in[:, og:, :])
            nc.scalar.dma_start(
                out=Lwi[:].rearrange("kk (i r) -> kk i r", r=128), in_=lhsi_in[:])

            e3v8 = E[:].rearrange("p (hh c b) -> p hh c b", c=8, b=B)
            e643v8 = E64[:].rearrange("p (hh c b) -> p hh c b", c=8, b=B)
            odA_v = odA[:].rearrange("p (m kb b) -> p m kb b", kb=NKB, b=B)
            odB_v = odB[:].rearrange("p (m kb b) -> p m kb b", kb=NKB, b=B)
            obi_v = obi[:].rearrange("p (kb w b) -> p kb w b", w=16, b=B)

            # slots: direct bands 0-11 in three 4-chunks, decim bands 12-15
            # (individual contiguous DMAs from obi), direct 12-15 -> slots
            # 16-19 individually. All out-DMAs ride the SP ring.
            chunk_map = {}
            for sl in range(12):
                chunk_map[sl] = (sl - sl % 4, 4)
            for sl in range(16, 20):
                chunk_map[sl] = (sl, 1)
            tiles = {}

            def slot_tile(slot):
                s0, n = chunk_map[slot]
                if s0 not in tiles:
                    tiles[s0] = opool.tile([128, n * 2048], F16, name="obc",
                                           tag=f"obc{s0}", bufs=1)
                return tiles[s0], (slot - s0) * 2048

            def ship_slot(slot):
                s0, n = chunk_map[slot]
                if slot == s0 + n - 1:
                    ob = tiles[s0]
                    nc.sync.dma_start(
                        out=out_t[s0 : s0 + n].rearrange("i p f -> p i f"),
                        in_=ob[:].rearrange("p (i f) -> p i f", i=n))

            def direct_band(di, group_outer=False):
                k = d_ord[di]
                t, q_cnt, s, use64, h_base = dirplan[k]
                o = d_off[di]
                src = E64 if use64 else E
                slot = di if di < 12 else 4 + di
                ob, obase = slot_tile(slot)
                if group_outer:
                    for g in range(GROUPS):
                        ps = ppool.tile([128, 512], F32, name="ps", tag="ps")
                        for qi in range(q_cnt):
                            h0 = 4 * g + h_base + qi
                            nc.tensor.matmul(
                                ps[:],
                                Lw[:, (o + qi) * 128 : (o + qi + 1) * 128],
                                src[:, h0 * 128 : h0 * 128 + 512],
                                start=(qi == 0), stop=(qi == q_cnt - 1))
                        base = obase + g * 512
                        nc.vector.tensor_copy(ob[:, base : base + 384],
                                              ps[:, 0:384])
                        nc.scalar.copy(ob[:, base + 384 : base + 512],
                                       ps[:, 384:512])
                else:
                    pss = []
                    for g in range(GROUPS):
                        ps_g = ppool.tile([128, 512], F32, name="ps", tag="ps")
                        pss.append(ps_g)
                    for qi in range(q_cnt):
                        w = Lw[:, (o + qi) * 128 : (o + qi + 1) * 128]
                        for g in range(GROUPS):
                            h0 = 4 * g + h_base + qi
                            nc.tensor.matmul(
                                pss[g][:], w,
                                src[:, h0 * 128 : h0 * 128 + 512],
                                start=(qi == 0), stop=(qi == q_cnt - 1))
                    for g in range(GROUPS):
                        base = obase + g * 512
                        nc.vector.tensor_copy(ob[:, base : base + 384],
                                              pss[g][:, 0:384])
                        nc.scalar.copy(ob[:, base + 384 : base + 512],
                                       pss[g][:, 384:512])
                ship_slot(slot)

            def dec_conv(kbi):
                k = dec_ord[kbi]
                t, s8, qd, use64, h0 = decplan[k]
                o = dec_off[kbi]
                v8 = e643v8 if use64 else e3v8
                ps = ppool.tile([128, 512], F32, name="ps", tag="ps")
                for qi in range(qd):
                    hh0, cq = divmod(h0 + qi, 8)
                    nc.tensor.matmul(
                        ps[:, 0:384],
                        Lwd[:, (o + qi) * 128 : (o + qi + 1) * 128],
                        v8[:, hh0 : hh0 + 3, cq, :],
                        start=(qi == 0), stop=(qi == qd - 1))
                nc.vector.tensor_copy(odA_v[:, 0:2, kbi, :], ps[:, 0:256])
                nc.scalar.copy(odA_v[:, 2:3, kbi, :], ps[:, 256:384])

            def interp_wgroup(wlist):
                """A few interp w-window matmuls + drains (diluted into the
                direct-band stream by the scheduler below)."""
                for w in wlist:
                    u_base = 64 * ((16 * w - 20) // 64)
                    c = 16 * w - u_base
                    assert c == CLS[w % 4], (w, c)
                    if u_base % 128 == 0:
                        rhs = odB_v[:, u_base // 128, :, :]
                    else:
                        rhs = odA_v[:, (u_base + 64) // 128, :, :]
                    ps = ppool.tile([128, 512], F32, name="ps", tag="ps")
                    nc.tensor.matmul(
                        ps[:, 0 : NKB * 128],
                        Lwi[:, (w % 4) * 128 : (w % 4) * 128 + 128],
                        rhs, start=True, stop=True)
                    nc.vector.tensor_copy(obi_v[:, 0:3, w : w + 1, :],
                                          ps[:, 0:384])
                    nc.scalar.copy(obi_v[:, 3:4, w : w + 1, :],
                                   ps[:, 384:512])

            def ship_decim():
                for j in range(NKB):
                    nc.sync.dma_start(
                        out=out_t[12 + j].rearrange("p (w b) -> p w b", b=B),
                        in_=obi_v[:, j, :, :])

            # ---- schedule: convs early (drain-light), interp w-groups
            # diluted between direct bands ----
            direct_band(0, group_outer=True)
            direct_band(1)
            for kbi in range(NKB):
                dec_conv(kbi)
            nc.sync.dma_start(out=odB_v[0:64, :, :, :], in_=odA_v[64:128, 0:2, :, :])
            nc.sync.dma_start(out=odB_v[64:128, :, :, :], in_=odA_v[0:64, 1:3, :, :])
            direct_band(2)
            direct_band(3)
            interp_wgroup([0, 1])
            direct_band(4)
            interp_wgroup([2, 3])
            direct_band(5)
            interp_wgroup([4, 5])
            direct_band(6)
            interp_wgroup([6, 7])
            direct_band(7)
            interp_wgroup([8, 9])
            direct_band(8)
            interp_wgroup([10, 11])
            direct_band(9)
            interp_wgroup([12, 13])
            direct_band(10)
            interp_wgroup([14, 15])
            direct_band(11)
            ship_decim()
            direct_band(12)
            direct_band(13)
            direct_band(14)
            direct_band(15)

    nc.compile()
    _program_cache[plan_key] = nc
    return nc


def _maybe_register_trace_hook():
    try:
        import sys
        import types

        import antenv

        if getattr(antenv, "axon_hooks", None) is not None:
            return
        from trn_agent_boot.trn_boot import _ntff_profile_via_ctypes

        hooks = types.ModuleType("antenv.axon_hooks")
        hook = _ntff_profile_via_ctypes("/opt/axon/libaxon_pjrt.so")
        hooks.get_axon_ntff_profile_hook = lambda: hook
        hooks.set_axon_ntff_profile_hook = lambda h: None
        antenv.axon_hooks = hooks
        sys.modules["antenv.axon_hooks"] = hooks
    except Exception:
        pass


def kernel(x: np.ndarray, kernels: np.ndarray, padlen) -> np.ndarray:
    global LAST_RESULT
    x = np.asarray(x, dtype=np.float32)
    kernels = np.asarray(kernels, dtype=np.float32)
    assert x.shape == (B, 1, L) and kernels.shape[0] == NB
    assert int(padlen) == P

    dirplan = _dir_plan(kernels)
    decplan = _dec_plan(kernels)
    plan_key = (tuple(sorted(dirplan.items())), tuple(sorted(decplan.items())))
    nc = _build_program(plan_key)
    d_ord, dec_ord = _orders(dirplan)

    lhs = np.ascontiguousarray(_dir_blocks(kernels, dirplan, d_ord))
    lhsd = np.ascontiguousarray(_dec_blocks(kernels, decplan, dec_ord))
    lhsi = np.ascontiguousarray(_interp_blocks())

    x2d = x[:, 0, :]
    left = 2.0 * x2d[:, :1] - x2d[:, 1 : EXTO + 1][:, ::-1]
    right = 2.0 * x2d[:, -1:] - x2d[:, -EXTO - 1 : -1][:, ::-1]
    ext_t = np.concatenate([left, x2d, right], axis=1).T.astype(np.float16)

    in_maps = []
    for c in range(N_CORES):
        sl = ext_t[c * LC : c * LC + EXT_ROWS]
        slp = np.ascontiguousarray(sl.reshape(H_E, 128, B).transpose(1, 0, 2))
        in_maps.append({"ext": slp, "lhs": lhs, "lhsd": lhsd, "lhsi": lhsi})

    trace = bool(os.environ.get("KERNEL_TRACE"))
    if trace:
        _maybe_register_trace_hook()
    res = run_bass_kernel_spmd(nc, in_maps, list(range(N_CORES)), trace=trace)
    LAST_RESULT = res

    slot_to_band = d_ord[0:12] + list(dec_ord) + d_ord[12:16]
    out = np.empty((B, 1, NB, L), np.float32)
    band_of_slot = np.asarray(slot_to_band)
    for c in range(N_CORES):
        dev = res.results[c]["out"].astype(np.float32).reshape(NB, 128, GROUPS, 4, 128)
        arr = dev.transpose(4, 0, 2, 3, 1).reshape(B, NB, LC)
        out[:, 0, band_of_slot, c * LC : (c + 1) * LC] = arr
    return out


# revision 45
# speedup vs baseline: 1.1138x; 1.0876x over previous
"""Trainium2 Bass kernel for batched filtfilt band-pass filtering (tensorpac-style).

Math: scipy-style filtfilt with FIR taps b is (exactly) a single convolution of
the odd-extended input with the autocorrelation of b, evaluated on the interior:

    out[n] = sum_d A[d] * ext[P + n + d],   d in [-(t-1), t-1]
    A[d]   = sum_i b[i] * b[i+d]            (t = effective tap count)

provided padlen P >= t-1 (true here: P = 512, t <= 513). The left "lfilter_zi"
constant extension and the right-edge extension of the backward pass never reach
the retained [P, P+L) window, so the equivalence is exact (verified to 1e-16).

Device mapping (per core, sequence-parallel over 8 cores):
  - each core owns 2048 output positions x all 128 batches; its input is a
    (3072, 128) slice of ext^T (position-major) covering the 2x512 halo,
    shipped fp16 in the SBUF-native [partition, h-block, batch] layout.
  - out[r, (j,b)] tiles (128 positions x 4 pos-blocks x 128 batches) accumulate
    in fp32 PSUM via K=128 fp16 matmuls: lhsT = 128x128 banded-Toeplitz blocks
    of A (host-precomputed fp16 constants), rhs = 512-wide slices of ext^T.
  - per band, the number of Toeplitz blocks adapts to the true tap support
    (Q = ceil((2t+126)/128)); a half-block-shifted copy of ext^T (E64, built
    on-device from E via two partition-shifted SBUF->SBUF DMAs) lets short
    bands cover their diagonal band with Q = minimal block count.
  - loop is band-outer (large/small-Q bands interleaved) so the per-band
    constant stream (2.1 MB) overlaps the matmul phase and the PSUM drain
    stays smooth; PSUM tiles drain via a DVE/ACT split copy that also casts
    to fp16, and each band leaves as one contiguous 0.5 MB DMA on the SP
    HWDGE ring (the last band streams per group to shorten the tail).
  - dummy warm-up matmuls run while the first inputs land so the PE HAM
    clock-gate is released before real work starts.
"""

import os

import numpy as np

import concourse.mybir as mybir
from concourse import bacc
from concourse.tile import TileContext
from concourse.bass_utils import run_bass_kernel_spmd

F32 = mybir.dt.float32
F16 = mybir.dt.float16

B = 128          # batch
L = 16384        # sequence length
P = 512          # padlen (= TAPS - 1)
NB = 20          # bands
N_CORES = 8
LC = L // N_CORES            # 2048 output positions per core
GROUPS = LC // 512           # 4 groups of 512 positions
EXT_ROWS = LC + 2 * P        # 3072 ext rows per core (halo included)
H_E = EXT_ROWS // 128        # 24 aligned 128-row blocks
H_E64 = (EXT_ROWS - 128) // 128  # 23 half-shifted blocks (rows 64 + 128h + p)
N_WARM = 14                  # dummy matmuls to warm the PE HAM during input DMA

LAST_RESULT = None  # BassKernelResults of the most recent run (for test harness)

_program_cache: dict = {}


def _band_plan(kernels: np.ndarray):
    """Per-band tap support -> (t, Q, s, use64, h_base) block plan.

    Block q covers ext rows m = n0 + P - s + 128q + kk (kk = partition), so
    diagonal d = 128q + kk - s - r. Coverage of d in [-(t-1), t-1] for every
    r in [0,128) requires s >= t-1 and s <= 128Q - 127 - t. s is the smallest
    multiple of 64 >= t-1; s % 128 == 64 uses the half-shifted E64 copy.
    """
    plan = []
    for k in range(kernels.shape[0]):
        nz = np.nonzero(kernels[k])[0]
        t = int(nz[-1]) + 1 if nz.size else 1
        assert t - 1 <= P, f"band {k}: taps {t} exceed padlen {P}"
        q_cnt = (2 * t + 126 + 127) // 128
        s = 64 * ((t - 1 + 63) // 64) if t > 1 else 0
        assert s >= t - 1 and s <= 128 * q_cnt - 127 - t, (k, t, q_cnt, s)
        use64 = (s % 128) == 64
        if use64:
            h_base = (P - 64 - s) // 128
        else:
            h_base = (P - s) // 128
        assert h_base >= 0
        plan.append((t, q_cnt, s, use64, h_base))
    return plan


def _band_order(plan):
    """First a small aligned band (so PE work starts on a partial E), LAST
    the biggest band: small-Q bands drain slower than they matmul (drain
    ~1.9us/band vs Q=2 matmul 1.7us), so each must sit next to a big band
    that gives DVE/ACT slack - ending on the biggest band lets every
    earlier drain catch up and keeps the PE gap-free to the end. E64 bands
    are kept out of the first two slots to cover the E64 build latency."""
    by_q = sorted(range(len(plan)), key=lambda k: (-plan[k][1], plan[k][3]))
    small_aligned = [k for k in by_q if not plan[k][3]]
    first = small_aligned[-1] if small_aligned else by_q[0]
    last = by_q[0] if by_q[0] != first else by_q[1]
    rest = [k for k in by_q if k != first and k != last]
    order, lo, hi = [first], 0, len(rest) - 1
    while lo <= hi:
        order.append(rest[lo]); lo += 1
        if lo <= hi:
            order.append(rest[hi]); hi -= 1
    order.append(last)
    return order


def _toeplitz_blocks(kernels: np.ndarray, plan, order):
    """Stacked lhsT blocks in SBUF-native layout: (128, NBLK, 128) fp16,
    [kk, block, r] with the contraction dim kk on axis 0. Blocks are laid
    out in BAND-ORDER (slot-major) so the whole constant stream is one or
    two contiguous DMAs that land in the order the matmuls consume them."""
    nblk = sum(p[1] for p in plan)
    out = np.zeros((128, nblk, 128), np.float16)
    kk = np.arange(128)[:, None]
    rr = np.arange(128)[None, :]
    i = 0
    for k in order:
        t, q_cnt, s, _use64, _hb = plan[k]
        bk = kernels[k][:t].astype(np.float64)
        acorr = np.correlate(bk, bk, mode="full")  # length 2t-1, center t-1
        a_full = np.zeros(2 * P + 1, np.float64)
        a_full[P - (t - 1) : P + t] = acorr
        for q in range(q_cnt):
            d = 128 * q - s + kk - rr
            valid = (d >= -(t - 1)) & (d <= t - 1)
            blk = np.where(valid, a_full[np.clip(d + P, 0, 2 * P)], 0.0)
            out[:, i, :] = blk.astype(np.float16)
            i += 1
    return out


def _build_program(plan_key):
    """Compile the SPMD program for a given block structure. Cached."""
    if plan_key in _program_cache:
        return _program_cache[plan_key]

    plan = list(plan_key)
    order = _band_order(plan)
    # block offsets are SLOT-major (band-order), matching _toeplitz_blocks
    slot_offsets = np.cumsum([0] + [plan[k][1] for k in order]).tolist()
    nblk = slot_offsets[-1]
    # out-DMA taper: leading slots ship in 4-band chunks (fewer ~0.6us
    # triggers on the sequencers), the last 8 slots ship individually the
    # moment they drain (4KB/partition descriptors, alternating rings) so
    # nothing big queues at the kernel tail
    chunk_sizes = (4, 4, 4, 1, 1, 1, 1, 1, 1, 1, 1)
    assert sum(chunk_sizes) == NB
    out_chunks = []
    si = 0
    for n in chunk_sizes:
        out_chunks.append((si, n))
        si += n

    nc = bacc.Bacc("TRN2", target_bir_lowering=False, debug=False,
                   num_devices=N_CORES)
    # host-permuted ext^T slice: [p, h, b] fp16 (SBUF-native layout)
    ext_in = nc.declare_dram_parameter("ext", [128, H_E, B], F16, isOutput=False)
    lhs_in = nc.declare_dram_parameter("lhs", [128, nblk, 128], F16,
                                       isOutput=False)
    out_t = nc.declare_dram_parameter("out", [NB, 128, GROUPS * 512], F16,
                                      isOutput=True)

    with TileContext(nc) as tc:
        with (
            tc.tile_pool(name="consts", bufs=1) as cpool,
            tc.tile_pool(name="psum", bufs=8, space="PSUM") as ppool,
            tc.tile_pool(name="ostage", bufs=6) as opool,
        ):
            E = cpool.tile([128, H_E * 128], F16)
            E64 = cpool.tile([128, H_E64 * 128], F16)
            Lw = cpool.tile([128, nblk * 128], F16)
            warm = cpool.tile([128, 256], F16)
            wps = ppool.tile([128, 512], F32, tag="ps")

            # PE warm-up during the input DMAs: harmless matmuls on a zeroed
            # tile keep the HAM busy window alive so real matmuls start warm.
            # memset on DVE: nc.any would pick GpSimd, whose multi-us engine
            # cold-start delays the whole warm-up chain.
            nc.vector.memset(warm[:], 0.0)
            for w in range(N_WARM):
                nc.tensor.matmul(wps[:, 0:256], warm[:, :128], warm[:],
                                 start=True, stop=True)

            # E in 3 asymmetric chunks: the first covers exactly the h-blocks
            # the first band's g=0 matmuls touch, so real matmuls start ASAP
            t0_, q0_, _s0, _u0, hb0 = plan[order[0]]
            chunk0 = min(hb0 + q0_ + 3, 15) * 128
            e_flat = ext_in[:].rearrange("p h b -> p (h b)")
            chunk = 15 * 128
            nc.sync.dma_start(out=E[:, 0:chunk0], in_=e_flat[:, 0:chunk0])
            if chunk0 < chunk:
                nc.sync.dma_start(out=E[:, chunk0:chunk], in_=e_flat[:, chunk0:chunk])
            nc.sync.dma_start(out=E[:, chunk:], in_=e_flat[:, chunk:])
            # E64[p, h] = ext rows (64 + 128h + p), built on device from E.
            # The sem-wait of these triggers head-of-line blocks the HWDGE
            # ring, which (deliberately) gives E exclusive DMA bandwidth.
            e3 = E[:].rearrange("p (h b) -> p h b", b=B)
            e643 = E64[:].rearrange("p (h b) -> p h b", b=B)
            nc.sync.dma_start(out=e643[0:64, :, :], in_=e3[64:128, 0:H_E64, :])
            nc.sync.dma_start(out=e643[64:128, :, :], in_=e3[0:64, 1 : H_E64 + 1, :])

            # constants are pre-ordered slot-major on the host, so the 2.1 MB
            # stream is FOUR contiguous graduated DMAs on the ACT HWDGE ring
            # (4 trigger issues instead of 20). Graduation matters because a
            # DMA completes as one unit: each chunk must land before the MM
            # stream reaches its first slot, so early chunks are small.
            for lo, hi in ((0, 2), (2, 6), (6, 12), (12, NB)):
                oa, ob_ = slot_offsets[lo], slot_offsets[hi]
                nc.scalar.dma_start(
                    out=Lw[:, oa * 128 : ob_ * 128].rearrange(
                        "kk (i r) -> kk i r", r=128
                    ),
                    in_=lhs_in[:, oa:ob_, :],
                )

            # chunk tiles staged for the tapered multi-band out-DMAs
            chunk_of_slot = {}
            for ci, (s0, n) in enumerate(out_chunks):
                for j in range(n):
                    chunk_of_slot[s0 + j] = ci
            chunk_tiles = {}

            for si, k in enumerate(order):
                t, q_cnt, s, use64, h_base = plan[k]
                o = slot_offsets[si]
                src = E64 if use64 else E
                h_max = H_E64 if use64 else H_E
                ci = chunk_of_slot[si]
                s0, n = out_chunks[ci]
                if ci not in chunk_tiles:
                    chunk_tiles[ci] = opool.tile(
                        [128, n * GROUPS * 512], F16, name="obc",
                        tag=f"obc{n}", bufs=(2 if n > 1 else 3),
                    )
                ob = chunk_tiles[ci]
                obase = (si - s0) * GROUPS * 512
                if k == order[0]:
                    # group-outer for the first band only: its g=0 matmuls
                    # need just the first few E h-blocks, so PE work starts
                    # while the rest of E is still in flight
                    for g in range(GROUPS):
                        ps = ppool.tile([128, 512], F32)
                        for qi in range(q_cnt):
                            h0 = 4 * g + h_base + qi
                            assert 0 <= h0 and h0 + 4 <= h_max, (k, g, qi, h0)
                            nc.tensor.matmul(
                                ps[:],
                                Lw[:, (o + qi) * 128 : (o + qi + 1) * 128],
                                src[:, h0 * 128 : h0 * 128 + 512],
                                start=(qi == 0),
                                stop=(qi == q_cnt - 1),
                            )
                        base = obase + g * 512
                        nc.vector.tensor_copy(ob[:, base : base + 384], ps[:, 0:384])
                        nc.scalar.copy(ob[:, base + 384 : base + 512], ps[:, 384:512])
                else:
                    # qi-outer: the 4 groups' PSUM banks accumulate in
                    # lockstep so each lhsT block is (re)used by 4 back-to-
                    # back matmuls - the weight (re)load amortizes across
                    # the group sweep instead of being paid per matmul
                    pss = []
                    for g in range(GROUPS):
                        ps_g = ppool.tile([128, 512], F32, name="ps", tag="ps")
                        pss.append(ps_g)
                    for qi in range(q_cnt):
                        w = Lw[:, (o + qi) * 128 : (o + qi + 1) * 128]
                        for g in range(GROUPS):
                            h0 = 4 * g + h_base + qi
                            assert 0 <= h0 and h0 + 4 <= h_max, (k, g, qi, h0)
                            nc.tensor.matmul(
                                pss[g][:],
                                w,
                                src[:, h0 * 128 : h0 * 128 + 512],
                                start=(qi == 0),
                                stop=(qi == q_cnt - 1),
                            )
                    # split the PSUM drain across DVE and ACT so neither
                    # engine gates the PSUM bank turnaround
                    for g in range(GROUPS):
                        base = obase + g * 512
                        nc.vector.tensor_copy(ob[:, base : base + 384], pss[g][:, 0:384])
                        nc.scalar.copy(ob[:, base + 384 : base + 512], pss[g][:, 384:512])
                # ship each completed chunk as ONE contiguous DMA (out_t is
                # slot-major; the host unscrambles), alternating rings
                # chunk-by-chunk. Keep the partition dim outermost on BOTH
                # sides of the AP - a leading free dim over SBUF partitions
                # generates descriptors the DGE cannot execute.
                if si == s0 + n - 1:
                    eng = nc.sync if ci % 2 == 0 else nc.scalar
                    eng.dma_start(
                        out=out_t[s0 : s0 + n].rearrange("i p f -> p i f"),
                        in_=ob[:].rearrange("p (i f) -> p i f", i=n),
                    )

    nc.compile()
    _program_cache[plan_key] = nc
    return nc


def _maybe_register_trace_hook():
    """Best-effort registration of the axon NTFF profile hook (profiling only;
    harmless no-op if unavailable)."""
    try:
        import sys
        import types

        import antenv

        if getattr(antenv, "axon_hooks", None) is not None:
            return
        from trn_agent_boot.trn_boot import _ntff_profile_via_ctypes

        hooks = types.ModuleType("antenv.axon_hooks")
        hook = _ntff_profile_via_ctypes("/opt/axon/libaxon_pjrt.so")
        hooks.get_axon_ntff_profile_hook = lambda: hook
        hooks.set_axon_ntff_profile_hook = lambda h: None
        antenv.axon_hooks = hooks
        sys.modules["antenv.axon_hooks"] = hooks
    except Exception:
        pass


def kernel(x: np.ndarray, kernels: np.ndarray, padlen) -> np.ndarray:
    global LAST_RESULT
    x = np.asarray(x, dtype=np.float32)
    kernels = np.asarray(kernels, dtype=np.float32)
    assert x.shape == (B, 1, L) and kernels.shape[0] == NB
    assert int(padlen) == P

    plan = _band_plan(kernels)
    plan_key = tuple(plan)
    nc = _build_program(plan_key)

    order = _band_order(plan)
    lhs = np.ascontiguousarray(_toeplitz_blocks(kernels, plan, order))

    # odd extension + transpose to position-major (ext^T), fp16
    x2d = x[:, 0, :]
    left = 2.0 * x2d[:, :1] - x2d[:, 1 : P + 1][:, ::-1]
    right = 2.0 * x2d[:, -1:] - x2d[:, -P - 1 : -1][:, ::-1]
    ext_t = np.concatenate([left, x2d, right], axis=1).T.astype(np.float16)

    in_maps = []
    for c in range(N_CORES):
        sl = ext_t[c * LC : c * LC + EXT_ROWS]  # (3072, B)
        # SBUF-native layout [p, h, b]: row (128h + p) -> [p, h]
        slp = np.ascontiguousarray(
            sl.reshape(H_E, 128, B).transpose(1, 0, 2)
        )
        in_maps.append({"ext": slp, "lhs": lhs})

    trace = bool(os.environ.get("KERNEL_TRACE"))
    if trace:
        _maybe_register_trace_hook()
    res = run_bass_kernel_spmd(nc, in_maps, list(range(N_CORES)), trace=trace)
    LAST_RESULT = res

    out = np.empty((B, 1, NB, L), np.float32)
    band_of_slot = np.asarray(order)  # out rows are slot-major on device
    for c in range(N_CORES):
        dev = res.results[c]["out"].astype(np.float32).reshape(NB, 128, GROUPS, 4, 128)
        # dev[slot, r, g, j, b] -> out[b, 0, order[slot], c*LC + 512g + 128j + r]
        arr = dev.transpose(4, 0, 2, 3, 1).reshape(B, NB, LC)
        out[:, 0, band_of_slot, c * LC : (c + 1) * LC] = arr
    return out

